# revision 1
# baseline (speedup 1.0000x reference)
"""Infini-attention decoder block on 8 Trainium2 NeuronCores.

Strategy (fully transposed activation layout: features on SBUF partitions,
tokens on the free dim):
  Phase A (head-parallel): core c owns heads {2c, 2c+1}. Streams x^T once,
    computes q^T/k^T (transposed) + v (natural) projections per (batch,
    segment), runs the compressive-memory scan per head with all the
    linear-attention algebra expressed as PE matmuls (ones-matmul broadcasts
    for partition-dim reductions, PE transposes for the sk layout flip).
  AllToAll: reshards attention output from head-sharded to token-sharded
    (64 MB bf16 total).
  Phase B (token-parallel): core c owns 2048 tokens. O-projection, fused
    rmsnorm+residual, SwiGLU FFN with streamed weights, final norm+residual.
Host pre-transposes/casts inputs per core and reassembles the output.
"""
import math

import numpy as np
import ml_dtypes

import concourse.bass as bass
import concourse.mybir as mybir
import concourse.tile as tile
from concourse import bacc
from concourse.bass_utils import run_bass_kernel_spmd

AF = mybir.ActivationFunctionType
ALU = mybir.AluOpType
BF16 = mybir.dt.bfloat16
FP32 = mybir.dt.float32
NPBF16 = ml_dtypes.bfloat16

N_CORES = 8


class Cfg:
    def __init__(self, B, S, E, H, DH, NSEG):
        self.B, self.S, self.E, self.H, self.DH, self.NSEG = B, S, E, H, DH, NSEG
        self.SEG = S // NSEG
        self.BS = B * S
        self.ECH = E // 128           # e-chunks
        self.HPC = H // N_CORES       # heads per core
        self.DPC = self.HPC * DH      # head dims per core
        self.NJC = self.SEG // 128    # 128-row chunks per segment
        self.TSL = self.BS // N_CORES  # tokens per core in phase B
        self.TBLK = min(512, self.TSL)
        self.NTB = self.TSL // self.TBLK
        self.F = 4 * E
        self.FCH = self.F // 128
        assert self.SEG % 128 == 0 and self.TSL % self.SEG == 0
        assert self.DPC % 128 == 0 and E % 128 == 0


FULL = Cfg(B=4, S=4096, E=2048, H=16, DH=128, NSEG=8)


def build(nc, cfg, g, flags, a2a=True, parts="AB"):
    """Emit the SPMD program. g = sigmoid(beta). flags: which bias/normw paths
    are active (compile-time; values are baked per-call)."""
    c = cfg
    E, DH, SEG, ECH, HPC, NJC, TSL, TBLK, NTB, FCH = (
        c.E, c.DH, c.SEG, c.ECH, c.HPC, c.NJC, c.TSL, c.TBLK, c.NTB, c.FCH)
    HDCH = E // 128
    scale = 1.0 / math.sqrt(DH)

    # ---------------- I/O ----------------
    XT = nc.dram_tensor("xT", [E, c.BS], BF16, kind="ExternalInput")
    XSL = nc.dram_tensor("xsl", [E, TSL], FP32, kind="ExternalInput")
    WQ = nc.dram_tensor("wq", [E, c.DPC], BF16, kind="ExternalInput")
    WK = nc.dram_tensor("wk", [E, c.DPC], BF16, kind="ExternalInput")
    WV = nc.dram_tensor("wv", [E, c.DPC], BF16, kind="ExternalInput")
    WO = nc.dram_tensor("wo", [E, E], BF16, kind="ExternalInput")
    W1 = nc.dram_tensor("w1", [E, c.F], BF16, kind="ExternalInput")
    W3 = nc.dram_tensor("w3", [E, c.F], BF16, kind="ExternalInput")
    W2 = nc.dram_tensor("w2", [c.F, E], BF16, kind="ExternalInput")
    OUT = nc.dram_tensor("outT", [E, TSL], FP32, kind="ExternalOutput")
    bio = {}
    if flags["bqkv"]:
        for nme in ("bq", "bk", "bv"):
            bio[nme] = nc.dram_tensor(nme, [128, c.DPC], FP32, kind="ExternalInput")
    if flags["bo"]:
        bio["bo"] = nc.dram_tensor("bo", [HDCH, 128], FP32, kind="ExternalInput")
    if flags["b13"]:
        bio["b1"] = nc.dram_tensor("b1", [FCH, 128], FP32, kind="ExternalInput")
        bio["b3"] = nc.dram_tensor("b3", [FCH, 128], FP32, kind="ExternalInput")
    if flags["b2"]:
        bio["b2"] = nc.dram_tensor("b2", [HDCH, 128], FP32, kind="ExternalInput")
    if flags["aw"]:
        bio["aw"] = nc.dram_tensor("aw", [HDCH, 128], FP32, kind="ExternalInput")
    if flags["fw"]:
        bio["fw"] = nc.dram_tensor("fw", [HDCH, 128], FP32, kind="ExternalInput")

    # inline constants
    tri_np = np.triu(np.ones((128, 128), np.float32)).astype(NPBF16)
    TRI = nc.inline_tensor(tri_np, name="tri")
    ONES = nc.inline_tensor(np.ones((128, 128), NPBF16), name="ones_c")
    OBLEND = nc.inline_tensor(
        np.full((128, 128), 1.0 / (1.0 - g), np.float32).astype(NPBF16), name="oblend")
    OK1 = nc.inline_tensor(np.ones((1, 128), NPBF16), name="ok1")
    IDENT = nc.inline_tensor(np.eye(128, dtype=np.float32).astype(NPBF16), name="ident")

    tc = tile.TileContext(nc, num_cores=N_CORES)
    with tc:
        with tc.tile_pool(name="dramp", bufs=1, space="DRAM") as dramp:
            a2a_in = dramp.tile([N_CORES, c.DPC, TSL], BF16)
            a2a_out = dramp.tile([N_CORES, c.DPC, TSL], BF16)

            # ======================= PHASE A =======================
            with (
                tc.tile_pool(name="acst", bufs=1) as acst,
                tc.tile_pool(name="awts", bufs=1) as awts,
                tc.tile_pool(name="asb", bufs=2) as asb,
                tc.tile_pool(name="astate", bufs=1) as astate,
                tc.tile_pool(name="apsA", bufs=5, space="PSUM") as apsA,
                tc.tile_pool(name="apsS", bufs=3, space="PSUM") as apsS,
            ):
                tri_sb = acst.tile([128, 128], BF16)
                nc.sync.dma_start(tri_sb[:], TRI[:])
                oblend_sb = acst.tile([128, 128], BF16)
                nc.sync.dma_start(oblend_sb[:], OBLEND[:])
                ok1_sb = acst.tile([1, 128], BF16)
                nc.sync.dma_start(ok1_sb[:], OK1[:])
                ident_sb = acst.tile([128, 128], BF16)
                nc.sync.dma_start(ident_sb[:], IDENT[:])

                wq_sb = awts.tile([128, ECH, c.DPC], BF16)
                nc.sync.dma_start(wq_sb[:], WQ[:].rearrange("(c p) d -> p c d", p=128))
                wk_sb = awts.tile([128, ECH, c.DPC], BF16)
                nc.sync.dma_start(wk_sb[:], WK[:].rearrange("(c p) d -> p c d", p=128))
                wv_sb = awts.tile([128, ECH, c.DPC], BF16)
                nc.sync.dma_start(wv_sb[:], WV[:].rearrange("(c p) d -> p c d", p=128))
                bqkv_sb = {}
                for nme in ("bq", "bk", "bv"):
                    if flags["bqkv"]:
                        t = awts.tile([128, c.DPC], FP32, name=f"{nme}_sb")
                        nc.sync.dma_start(t[:], bio[nme][:])
                        bqkv_sb[nme] = t

                for b in range(c.B if "A" in parts else 0):
                    sts = []
                    stbs = []
                    for h in range(HPC):
                        st = astate.tile([128, DH + 1], FP32, name=f"st_{b}_{h}",
                                         tag=f"st{h}", bufs=2)
                        sts.append(st)
                        stb = astate.tile([128, DH + 1], BF16, name=f"stb_{b}_{h}",
                                          tag=f"stb{h}", bufs=2)
                        stbs.append(stb)
                    for seg in range(c.NSEG):
                        t0 = b * c.S + seg * SEG
                        xt = asb.tile([128, ECH, SEG], BF16, tag="xt")
                        nc.sync.dma_start(
                            xt[:],
                            XT[:].rearrange("(c p) t -> p c t", p=128)[:, :, t0:t0 + SEG])

                        qT = asb.tile([128, HPC * SEG], BF16, tag="qT")
                        kT = asb.tile([128, HPC * SEG], BF16, tag="kT")
                        sqT = asb.tile([128, HPC * SEG], BF16, tag="sqT")
                        skT = asb.tile([128, HPC * SEG], BF16, tag="skT")
                        for (W_sb, dst, elu_dst, bkey) in (
                                (wq_sb, qT, sqT, "bq"), (wk_sb, kT, skT, "bk")):
                            for dt in range(HPC):
                                ps = apsA.tile([128, SEG], FP32, tag="pA", name="ps_qk")
                                for ec in range(ECH):
                                    nc.tensor.matmul(
                                        ps[:], W_sb[:, ec, dt * 128:(dt + 1) * 128],
                                        xt[:, ec, :],
                                        start=(ec == 0), stop=(ec == ECH - 1))
                                sl = slice(dt * SEG, (dt + 1) * SEG)
                                if flags["bqkv"]:
                                    nc.vector.tensor_scalar_add(
                                        ps[:], ps[:],
                                        bqkv_sb[bkey][:, dt * 128:dt * 128 + 1])
                                nc.scalar.copy(dst[:, sl], ps[:])
                                ex = asb.tile([128, SEG], BF16, tag="elu_ex")
                                nc.scalar.activation(ex[:], ps[:], AF.Exp)
                                rl = asb.tile([128, SEG], BF16, tag="elu_rl")
                                nc.vector.tensor_scalar_max(rl[:], ps[:], 0.0)
                                nc.vector.scalar_tensor_tensor(
                                    elu_dst[:, sl], ex[:], 1.0, rl[:],
                                    op0=ALU.min, op1=ALU.add)
                        v_tiles = []
                        for tt in range(NJC):
                            ps = apsA.tile([128, c.DPC], FP32, tag="pA", name="ps_v")
                            for ec in range(ECH):
                                nc.tensor.matmul(
                                    ps[:], xt[:, ec, tt * 128:(tt + 1) * 128],
                                    wv_sb[:, ec, :],
                                    start=(ec == 0), stop=(ec == ECH - 1))
                            if flags["bqkv"]:
                                vb = asb.tile([128, c.DPC], FP32, tag="vb")
                                nc.vector.tensor_add(vb[:], ps[:], bqkv_sb["bv"][:])
                                vt = asb.tile([128, c.DPC], BF16, tag="vnat", bufs=2 * NJC)
                                nc.scalar.copy(vt[:], vb[:])
                            else:
                                vt = asb.tile([128, c.DPC], BF16, tag="vnat", bufs=2 * NJC)
                                nc.scalar.copy(vt[:], ps[:])
                            v_tiles.append(vt)

                        for h in range(HPC):
                            hs = slice(h * SEG, (h + 1) * SEG)
                            st, stb = sts[h], stbs[h]
                            if seg > 0:
                                nc.vector.tensor_copy(stb[:], st[:])
                            # scores -> exp -> mask
                            expP = asb.tile([128, NJC, SEG], BF16, tag="expP")
                            for jc in range(NJC):
                                ps = apsA.tile([128, SEG], FP32, tag="pA", name="ps_s")
                                nc.tensor.matmul(
                                    ps[:], kT[:, h * SEG + jc * 128: h * SEG + (jc + 1) * 128],
                                    qT[:, hs], start=True, stop=True)
                                nc.scalar.activation(expP[:, jc, :], ps[:], AF.Exp,
                                                     scale=scale)
                                if jc > 0:
                                    nc.vector.memset(expP[:, jc, 0:jc * 128], 0.0)
                                nc.vector.tensor_mul(
                                    expP[:, jc, jc * 128:(jc + 1) * 128],
                                    expP[:, jc, jc * 128:(jc + 1) * 128], tri_sb[:])
                            # srow (scaled by 1/(1-g)) broadcast
                            ps_r = apsA.tile([128, SEG], FP32, tag="pA", name="ps_r")
                            for jc in range(NJC):
                                nc.tensor.matmul(ps_r[:], oblend_sb[:], expP[:, jc, :],
                                                 start=(jc == 0), stop=(jc == NJC - 1))
                            r1 = asb.tile([128, SEG], FP32, tag="r1")
                            nc.vector.reciprocal_approx_fast(r1[:], ps_r[:])
                            # A_dot unnormalized
                            ps_a = apsA.tile([128, SEG], FP32, tag="pA", name="ps_a")
                            for jc in range(NJC):
                                nc.tensor.matmul(
                                    ps_a[:], v_tiles[jc][:, h * DH:(h + 1) * DH],
                                    expP[:, jc, :],
                                    start=(jc == 0), stop=(jc == NJC - 1))
                            outT = asb.tile([128, SEG], BF16, tag="outT")
                            if seg == 0:
                                nc.vector.tensor_mul(outT[:], ps_a[:], r1[:])
                            else:
                                # A_mem unnormalized + den_q
                                ps_m = apsA.tile([128, SEG], FP32, tag="pA", name="ps_m")
                                nc.tensor.matmul(ps_m[:], stb[:, 0:DH], sqT[:, hs],
                                                 start=True, stop=True)
                                ps_d = apsS.tile([1, SEG], FP32, tag="pS", name="ps_d")
                                nc.tensor.matmul(ps_d[:], stb[:, DH:DH + 1], sqT[:, hs],
                                                 start=True, stop=True)
                                dq = asb.tile([1, SEG], BF16, tag="dq")
                                nc.vector.tensor_scalar(
                                    dq[:], ps_d[:], 1.0 / g, 1e-6 / g,
                                    op0=ALU.mult, op1=ALU.add)
                                ps_db = apsA.tile([128, SEG], FP32, tag="pA", name="ps_db")
                                nc.tensor.matmul(ps_db[:], ok1_sb[:], dq[:],
                                                 start=True, stop=True)
                                r2 = asb.tile([128, SEG], FP32, tag="r2")
                                nc.vector.reciprocal_approx_fast(r2[:], ps_db[:])
                                tmb = asb.tile([128, SEG], FP32, tag="tmb")
                                nc.vector.tensor_mul(tmb[:], ps_m[:], r2[:])
                                tab = asb.tile([128, SEG], FP32, tag="tab")
                                nc.vector.tensor_mul(tab[:], ps_a[:], r1[:])
                                nc.vector.tensor_add(outT[:], tmb[:], tab[:])
                            # v_delta (negated) + ones col, then state update
                            ps_u = apsS.tile([128, DH + 1], FP32, tag="pS", name="ps_u")
                            for jc in range(NJC):
                                vd = asb.tile([128, DH + 1], BF16, tag="vd")
                                vch = v_tiles[jc][:, h * DH:(h + 1) * DH]
                                if seg == 0:
                                    nc.vector.tensor_scalar_mul(vd[:, 0:DH], vch, -1.0)
                                else:
                                    ps_t = apsS.tile([128, DH + 1], FP32, tag="pS",
                                                     name="ps_t")
                                    nc.tensor.matmul(
                                        ps_t[:],
                                        skT[:, h * SEG + jc * 128:h * SEG + (jc + 1) * 128],
                                        stb[:], start=True, stop=True)
                                    dkr = asb.tile([128, 1], FP32, tag="dkr")
                                    nc.vector.tensor_scalar_add(
                                        dkr[:], ps_t[:, DH:DH + 1], 1e-6)
                                    rk = asb.tile([128, 1], FP32, tag="rk")
                                    nc.vector.reciprocal_approx_fast(rk[:], dkr[:])
                                    nc.vector.scalar_tensor_tensor(
                                        vd[:, 0:DH], ps_t[:, 0:DH], rk[:], vch,
                                        op0=ALU.mult, op1=ALU.subtract)
                                nc.vector.memset(vd[:, DH:DH + 1], -1.0)
                                # sk_nat via PE transpose
                                ps_tr = apsS.tile([128, 128], BF16, tag="pS", name="ps_tr")
                                nc.tensor.transpose(
                                    ps_tr[:],
                                    skT[:, h * SEG + jc * 128:h * SEG + (jc + 1) * 128],
                                    ident_sb[:])
                                skn = asb.tile([128, 128], BF16, tag="skn")
                                nc.scalar.copy(skn[:], ps_tr[:])
                                nc.tensor.matmul(ps_u[:], skn[:], vd[:],
                                                 start=(jc == 0), stop=(jc == NJC - 1))
                            if seg == 0:
                                nc.vector.tensor_scalar_mul(st[:], ps_u[:], -1.0)
                            else:
                                nc.vector.tensor_sub(st[:], st[:], ps_u[:])
                            # stage attention output for A2A
                            tglob = b * c.S + seg * SEG
                            shard = tglob // TSL
                            off = tglob % TSL
                            nc.sync.dma_start(
                                a2a_in[shard, h * DH:(h + 1) * DH, off:off + SEG],
                                outT[:])

            if a2a:
                nc.gpsimd.collective_compute(
                    "AllToAll", ALU.bypass,
                    replica_groups=[list(range(N_CORES))],
                    ins=[a2a_in.opt()], outs=[a2a_out.opt()])
            else:
                a2a_out = a2a_in

            # ======================= PHASE B =======================
            with (
                tc.tile_pool(name="bcst", bufs=1) as bcst,
                tc.tile_pool(name="bkeep", bufs=2) as bkeep,
                tc.tile_pool(name="bpsB", bufs=4, space="PSUM") as bpsB,
                tc.tile_pool(name="bpsS", bufs=2, space="PSUM") as bpsS,
            ):
                ones_sb = bcst.tile([128, 128], BF16)
                nc.sync.dma_start(ones_sb[:], ONES[:])
                eps_sb = bcst.tile([128, 1], FP32)
                nc.vector.memset(eps_sb[:], 1e-5)
                bias_sb = {}
                for nme in ("bo", "b2", "aw", "fw", "b1", "b3"):
                    if nme in bio:
                        t = bcst.tile(list(bio[nme].shape), FP32, name=f"{nme}_sb")
                        nc.sync.dma_start(t[:], bio[nme][:])
                        bias_sb[nme] = t

                for tb in range(NTB if "B" in parts else 0):
                    ts0 = tb * TBLK
                    b1_cm = tc.tile_pool(name=f"b1_{tb}", bufs=2)
                    b1 = b1_cm.__enter__()
                    aT = b1.tile([128, HDCH, TBLK], BF16, tag="aT", bufs=1)
                    nc.sync.dma_start(
                        aT[:],
                        a2a_out[:].rearrange("s (q p) t -> p (s q) t", p=128)[
                            :, :, ts0:ts0 + TBLK])
                    # ---- O-projection + rmsnorm1 stats ----
                    ps_ss = bpsS.tile([128, TBLK], FP32, tag="pSt", name="ps_ss")
                    y_bf = []
                    for et in range(HDCH):
                        wo_t = b1.tile([128, HDCH, 128], BF16, tag="wo")
                        nc.sync.dma_start(
                            wo_t[:],
                            WO[:, et * 128:(et + 1) * 128].rearrange(
                                "(q p) e -> p q e", p=128))
                        ps = bpsB.tile([128, TBLK], FP32, tag="pB", name="ps_y")
                        for hc in range(HDCH):
                            nc.tensor.matmul(ps[:], wo_t[:, hc, :], aT[:, hc, :],
                                             start=(hc == 0), stop=(hc == HDCH - 1))
                        if flags["bo"]:
                            nc.vector.tensor_scalar_add(
                                ps[:], ps[:], bias_sb["bo"][et, :].reshape([128, 1]))
                        yb = b1.tile([128, TBLK], BF16, tag="ybf", bufs=HDCH + 1)
                        nc.scalar.copy(yb[:], ps[:])
                        y_bf.append(yb)
                        ysq = b1.tile([128, TBLK], BF16, tag="ysq")
                        nc.scalar.activation(ysq[:], ps[:], AF.Square)
                        nc.tensor.matmul(ps_ss[:], ones_sb[:], ysq[:],
                                         start=(et == 0), stop=(et == HDCH - 1))
                    sqs = b1.tile([128, TBLK], FP32, tag="sqs")
                    nc.scalar.activation(sqs[:], ps_ss[:], AF.Sqrt,
                                         scale=1.0 / E, bias=eps_sb[:])
                    rstd = b1.tile([128, TBLK], FP32, tag="rstd")
                    nc.vector.reciprocal_approx_fast(rstd[:], sqs[:])
                    # ---- x1 = x + n1 ----
                    n1_t = []
                    x1_t = []
                    for et in range(HDCH):
                        n1 = bkeep.tile([128, TBLK], BF16, tag="n1", bufs=HDCH + 1)
                        nc.vector.tensor_mul(n1[:], y_bf[et][:], rstd[:])
                        if flags["aw"]:
                            nc.vector.tensor_scalar_mul(
                                n1[:], n1[:], bias_sb["aw"][et, :].reshape([128, 1]))
                        n1_t.append(n1)
                        xs = b1.tile([128, TBLK], FP32, tag="xs")
                        nc.sync.dma_start(xs[:], XSL[et * 128:(et + 1) * 128,
                                                     ts0:ts0 + TBLK])
                        x1 = bkeep.tile([128, TBLK], BF16, tag="x1b", bufs=HDCH + 1)
                        nc.vector.tensor_add(x1[:], xs[:], n1[:])
                        x1_t.append(x1)
                    b1_cm.__exit__(None, None, None)
                    # ---- FFN up + gate ----
                    b2_cm = tc.tile_pool(name=f"b2_{tb}", bufs=2)
                    b2 = b2_cm.__enter__()
                    h_t = []
                    for ft in range(FCH):
                        w1_t = b2.tile([128, ECH, 128], BF16, tag="w1")
                        nc.sync.dma_start(
                            w1_t[:],
                            W1[:, ft * 128:(ft + 1) * 128].rearrange(
                                "(c p) f -> p c f", p=128))
                        w3_t = b2.tile([128, ECH, 128], BF16, tag="w3")
                        nc.sync.dma_start(
                            w3_t[:],
                            W3[:, ft * 128:(ft + 1) * 128].rearrange(
                                "(c p) f -> p c f", p=128))
                        ps1 = bpsB.tile([128, TBLK], FP32, tag="pB", name="ps_h1")
                        ps3 = bpsB.tile([128, TBLK], FP32, tag="pB", name="ps_h3")
                        for ec in range(ECH):
                            nc.tensor.matmul(ps1[:], w1_t[:, ec, :], x1_t[ec][:],
                                             start=(ec == 0), stop=(ec == ECH - 1))
                        for ec in range(ECH):
                            nc.tensor.matmul(ps3[:], w3_t[:, ec, :], x1_t[ec][:],
                                             start=(ec == 0), stop=(ec == ECH - 1))
                        g1 = b2.tile([128, TBLK], BF16, tag="g1")
                        if flags["b13"]:
                            nc.scalar.activation(
                                g1[:], ps1[:], AF.Gelu,
                                bias=bias_sb["b1"][ft, :].reshape([128, 1]))
                        else:
                            nc.scalar.activation(g1[:], ps1[:], AF.Gelu)
                        ht = b2.tile([128, TBLK], BF16, tag="h", bufs=FCH)
                        if flags["b13"]:
                            nc.vector.scalar_tensor_tensor(
                                ht[:], ps3[:], bias_sb["b3"][ft, :].reshape([128, 1]),
                                g1[:], op0=ALU.add, op1=ALU.mult)
                        else:
                            nc.vector.tensor_mul(ht[:], ps3[:], g1[:])
                        h_t.append(ht)
                    # ---- FFN down + rmsnorm2 + out ----
                    ps_ss2 = bpsS.tile([128, TBLK], FP32, tag="pSt", name="ps_ss2")
                    y2_bf = []
                    for et in range(HDCH):
                        ps = bpsB.tile([128, TBLK], FP32, tag="pB", name="ps_y2")
                        for half in range(2):
                            w2_t = b2.tile([128, FCH // 2, 128], BF16, tag="w2", bufs=2)
                            nc.sync.dma_start(
                                w2_t[:],
                                W2[half * (c.F // 2):(half + 1) * (c.F // 2),
                                   et * 128:(et + 1) * 128].rearrange(
                                    "(q p) e -> p q e", p=128))
                            for fq in range(FCH // 2):
                                fc = half * (FCH // 2) + fq
                                nc.tensor.matmul(
                                    ps[:], w2_t[:, fq, :], h_t[fc][:],
                                    start=(fc == 0), stop=(fc == FCH - 1))
                        if flags["b2"]:
                            nc.vector.tensor_scalar_add(
                                ps[:], ps[:], bias_sb["b2"][et, :].reshape([128, 1]))
                        y2 = b2.tile([128, TBLK], BF16, tag="y2bf", bufs=HDCH + 1)
                        nc.scalar.copy(y2[:], ps[:])
                        y2_bf.append(y2)
                        y2sq = b2.tile([128, TBLK], BF16, tag="y2sq")
                        nc.scalar.activation(y2sq[:], ps[:], AF.Square)
                        nc.tensor.matmul(ps_ss2[:], ones_sb[:], y2sq[:],
                                         start=(et == 0), stop=(et == HDCH - 1))
                    sqs2 = b2.tile([128, TBLK], FP32, tag="sqs2")
                    nc.scalar.activation(sqs2[:], ps_ss2[:], AF.Sqrt,
                                         scale=1.0 / E, bias=eps_sb[:])
                    rstd2 = b2.tile([128, TBLK], FP32, tag="rstd2")
                    nc.vector.reciprocal_approx_fast(rstd2[:], sqs2[:])
                    for et in range(HDCH):
                        n2 = b2.tile([128, TBLK], BF16, tag="n2")
                        nc.vector.tensor_mul(n2[:], y2_bf[et][:], rstd2[:])
                        if flags["fw"]:
                            nc.vector.tensor_scalar_mul(
                                n2[:], n2[:], bias_sb["fw"][et, :].reshape([128, 1]))
                        nn = b2.tile([128, TBLK], FP32, tag="nn")
                        nc.vector.tensor_add(nn[:], n1_t[et][:], n2[:])
                        xs2 = b2.tile([128, TBLK], FP32, tag="xs2")
                        nc.sync.dma_start(xs2[:], XSL[et * 128:(et + 1) * 128,
                                                      ts0:ts0 + TBLK])
                        of = b2.tile([128, TBLK], FP32, tag="of")
                        nc.vector.tensor_add(of[:], nn[:], xs2[:])
                        nc.sync.dma_start(OUT[et * 128:(et + 1) * 128, ts0:ts0 + TBLK],
                                          of[:])
                    b2_cm.__exit__(None, None, None)
    return tc


def _prep_inputs(inputs, cfg):
    """Host-side: slice/transpose/cast per core. Returns (in_maps, g, flags)."""
    c = cfg
    f32 = np.float32
    x = np.ascontiguousarray(np.asarray(inputs["x"], f32).reshape(c.BS, c.E))
    beta = float(np.asarray(inputs["beta"]).reshape(-1)[0])
    g = 1.0 / (1.0 + math.exp(-beta))
    g = min(max(g, 1e-6), 1.0 - 1e-6)

    Wq = np.asarray(inputs["Wq"], f32)
    Wk = np.asarray(inputs["Wk"], f32)
    Wv = np.asarray(inputs["Wv"], f32)
    Wo = np.asarray(inputs["Wo"], f32)
    W1 = np.asarray(inputs["W1"], f32)
    W2 = np.asarray(inputs["W2"], f32)
    W3 = np.asarray(inputs["W3"], f32)
    aw = np.asarray(inputs["attn_norm_w"], f32)
    fw = np.asarray(inputs["ffn_norm_w"], f32)
    bq = np.asarray(inputs["bq"], f32)
    bk = np.asarray(inputs["bk"], f32)
    bv = np.asarray(inputs["bv"], f32)
    bo = np.asarray(inputs["bo"], f32)
    b1 = np.asarray(inputs["b1"], f32)
    b2 = np.asarray(inputs["b2"], f32)
    b3 = np.asarray(inputs["b3"], f32)

    flags = {
        "bqkv": bool(np.any(bq) or np.any(bk) or np.any(bv)),
        "bo": bool(np.any(bo)),
        "b13": bool(np.any(b1) or np.any(b3)),
        "b2": bool(np.any(b2)),
        "aw": not bool(np.all(aw == 1.0)),
        "fw": not bool(np.all(fw == 1.0)),
    }

    xT = np.ascontiguousarray(x.T).astype(NPBF16)
    woT = np.ascontiguousarray(Wo.T).astype(NPBF16)
    w1T = np.ascontiguousarray(W1.T).astype(NPBF16)
    w3T = np.ascontiguousarray(W3.T).astype(NPBF16)
    w2T = np.ascontiguousarray(W2.T).astype(NPBF16)

    in_maps = []
    for core in range(N_CORES):
        ds = slice(core * c.DPC, (core + 1) * c.DPC)
        tok = slice(core * c.TSL, (core + 1) * c.TSL)
        m = {
            "xT": xT,
            "xsl": np.ascontiguousarray(x[tok].T),
            "wq": np.ascontiguousarray(Wq[ds].T).astype(NPBF16),
            "wk": np.ascontiguousarray(Wk[ds].T).astype(NPBF16),
            "wv": np.ascontiguousarray(Wv[ds].T).astype(NPBF16),
            "wo": woT, "w1": w1T, "w3": w3T, "w2": w2T,
        }
        if flags["bqkv"]:
            m["bq"] = np.tile(bq[ds][None, :], (128, 1)).astype(f32)
            m["bk"] = np.tile(bk[ds][None, :], (128, 1)).astype(f32)
            m["bv"] = np.tile(bv[ds][None, :], (128, 1)).astype(f32)
        if flags["bo"]:
            m["bo"] = bo.reshape(c.E // 128, 128).astype(f32)
        if flags["b13"]:
            m["b1"] = b1.reshape(c.FCH, 128).astype(f32)
            m["b3"] = b3.reshape(c.FCH, 128).astype(f32)
        if flags["b2"]:
            m["b2"] = b2.reshape(c.E // 128, 128).astype(f32)
        if flags["aw"]:
            m["aw"] = aw.reshape(c.E // 128, 128).astype(f32)
        if flags["fw"]:
            m["fw"] = fw.reshape(c.E // 128, 128).astype(f32)
        in_maps.append(m)
    return in_maps, g, flags


def run(inputs, cfg, trace=False):
    in_maps, g, flags = _prep_inputs(inputs, cfg)
    nc = bacc.Bacc("TRN2", target_bir_lowering=False, debug=False,
                   num_devices=N_CORES)
    build(nc, cfg, g, flags)
    nc.compile()
    res = run_bass_kernel_spmd(nc, in_maps, list(range(N_CORES)), trace=trace)
    out = np.empty((cfg.BS, cfg.E), np.float32)
    for core in range(N_CORES):
        out[core * cfg.TSL:(core + 1) * cfg.TSL] = res.results[core]["outT"].T
    return out.reshape(cfg.B, cfg.S, cfg.E), res


def kernel(**inputs):
    out, _ = run(inputs, FULL)
    return out



# revision 7
# speedup vs baseline: 1.0126x; 1.0126x over previous
"""Infini-attention decoder block on 8 Trainium2 NeuronCores.

Strategy (fully transposed activation layout: features on SBUF partitions,
tokens on the free dim):
  Phase A (head-parallel): core c owns heads {2c, 2c+1}. Streams x^T once,
    computes q^T/k^T (transposed) + v (natural) projections per (segment,
    batch) — seg-outer so the compressive-memory scan states for all 4
    batches advance together — with all the linear-attention algebra
    expressed as PE matmuls. Causally-masked column blocks of the local
    softmax are never computed (partial-N matmuls).
  AllToAll (x4): the attention output is resharded head->token in four
    token-quarter collectives; quarter q's last producer is segment q+4,
    so the collectives overlap the tail of phase A and phase B block 0
    starts with its data already resident.
  Phase B (token-parallel): core c owns 2048 tokens. O-projection, fused
    rmsnorm+residual, SwiGLU FFN with streamed weights, final norm+residual.
    Persistent tile pools let consecutive 512-token blocks overlap.
Host pre-transposes/casts inputs per core and reassembles the output.
"""
import math

import numpy as np
import ml_dtypes

import concourse.bass as bass
import concourse.mybir as mybir
import concourse.tile as tile
from concourse import bacc
from concourse.bass_utils import run_bass_kernel_spmd

AF = mybir.ActivationFunctionType
ALU = mybir.AluOpType
BF16 = mybir.dt.bfloat16
FP32 = mybir.dt.float32
NPBF16 = ml_dtypes.bfloat16

N_CORES = 8


class Cfg:
    def __init__(self, B, S, E, H, DH, NSEG):
        self.B, self.S, self.E, self.H, self.DH, self.NSEG = B, S, E, H, DH, NSEG
        self.SEG = S // NSEG
        self.BS = B * S
        self.ECH = E // 128           # e-chunks
        self.HPC = H // N_CORES       # heads per core
        self.DPC = self.HPC * DH      # head dims per core
        self.NJC = self.SEG // 128    # 128-row chunks per segment
        self.TSL = self.BS // N_CORES  # tokens per core in phase B
        self.TBLK = min(512, self.TSL)
        self.NTB = self.TSL // self.TBLK
        self.F = 4 * E
        self.FCH = self.F // 128
        assert self.SEG % 128 == 0 and self.TSL % self.SEG == 0
        assert self.DPC % 128 == 0 and E % 128 == 0


FULL = Cfg(B=4, S=4096, E=2048, H=16, DH=128, NSEG=8)


def build(nc, cfg, g, flags, a2a=True, parts="AB"):
    """Emit the SPMD program. g = sigmoid(beta). flags: which bias/normw paths
    are active (compile-time; values are baked per-call)."""
    c = cfg
    E, DH, SEG, ECH, HPC, NJC, TSL, TBLK, NTB, FCH = (
        c.E, c.DH, c.SEG, c.ECH, c.HPC, c.NJC, c.TSL, c.TBLK, c.NTB, c.FCH)
    HDCH = E // 128
    scale = 1.0 / math.sqrt(DH)
    NQ = NTB  # token quarters == phase-B blocks
    assert SEG == TBLK and c.NSEG == 2 * NQ

    # ---------------- I/O ----------------
    XT = nc.dram_tensor("xT", [E, c.BS], BF16, kind="ExternalInput")
    XSL = nc.dram_tensor("xslb", [E, TSL], BF16, kind="ExternalInput")
    WQ = nc.dram_tensor("wq", [E, c.DPC], BF16, kind="ExternalInput")
    WK = nc.dram_tensor("wk", [E, c.DPC], BF16, kind="ExternalInput")
    WV = nc.dram_tensor("wv", [E, c.DPC], BF16, kind="ExternalInput")
    WO = nc.dram_tensor("wo", [E, E], BF16, kind="ExternalInput")
    W1 = nc.dram_tensor("w1", [E, c.F], BF16, kind="ExternalInput")
    W3 = nc.dram_tensor("w3", [E, c.F], BF16, kind="ExternalInput")
    W2 = nc.dram_tensor("w2", [c.F, E], BF16, kind="ExternalInput")
    OUT = nc.dram_tensor("outT", [E, TSL], FP32, kind="ExternalOutput")
    bio = {}
    if flags["bqkv"]:
        for nme in ("bq", "bk", "bv"):
            bio[nme] = nc.dram_tensor(nme, [128, c.DPC], FP32, kind="ExternalInput")
    if flags["bo"]:
        bio["bo"] = nc.dram_tensor("bo", [HDCH, 128], FP32, kind="ExternalInput")
    if flags["b13"]:
        bio["b1"] = nc.dram_tensor("b1", [FCH, 128], FP32, kind="ExternalInput")
        bio["b3"] = nc.dram_tensor("b3", [FCH, 128], FP32, kind="ExternalInput")
    if flags["b2"]:
        bio["b2"] = nc.dram_tensor("b2", [HDCH, 128], FP32, kind="ExternalInput")
    if flags["aw"]:
        bio["aw"] = nc.dram_tensor("aw", [HDCH, 128], FP32, kind="ExternalInput")
    if flags["fw"]:
        bio["fw"] = nc.dram_tensor("fw", [HDCH, 128], FP32, kind="ExternalInput")

    # inline constants
    tri_np = np.triu(np.ones((128, 128), np.float32)).astype(NPBF16)
    TRI = nc.inline_tensor(tri_np, name="tri")
    ONES = nc.inline_tensor(np.ones((128, 128), NPBF16), name="ones_c")
    OBLEND = nc.inline_tensor(
        np.full((128, 128), 1.0 / (1.0 - g), np.float32).astype(NPBF16), name="oblend")
    OK1 = nc.inline_tensor(np.ones((1, 128), NPBF16), name="ok1")
    IDENT = nc.inline_tensor(np.eye(128, dtype=np.float32).astype(NPBF16), name="ident")

    tc = tile.TileContext(nc, num_cores=N_CORES)
    with tc:
        with tc.tile_pool(name="dramp", bufs=1, space="DRAM") as dramp:
            a2a_in = [dramp.tile([N_CORES, c.DPC, TBLK], BF16, name=f"a2ai{q}")
                      for q in range(NQ)]
            if a2a:
                a2a_out = [dramp.tile([N_CORES, c.DPC, TBLK], BF16,
                                      name=f"a2ao{q}") for q in range(NQ)]
            else:
                a2a_out = a2a_in

            # Persistent pool for phase-B stage 1 (O-proj / norm1): opened
            # before phase A so its tiles do not alias phase-A space and
            # block 0 can overlap phase A's tail.
            b1_cm = tc.tile_pool(name="bpb1", bufs=1)
            b1 = b1_cm.__enter__()
            ones_sb = b1.tile([128, 128], BF16, name="ones_sb")
            nc.sync.dma_start(ones_sb[:], ONES[:])
            eps_sb = b1.tile([128, 1], FP32, name="eps_sb")
            nc.vector.memset(eps_sb[:], 1e-5)
            bias_sb = {}
            for nme in ("bo", "b2", "aw", "fw", "b1", "b3"):
                if nme in bio:
                    t = b1.tile(list(bio[nme].shape), FP32, name=f"{nme}_sb")
                    nc.sync.dma_start(t[:], bio[nme][:])
                    bias_sb[nme] = t

            # ======================= PHASE A =======================
            with (
                tc.tile_pool(name="acst", bufs=1) as acst,
                tc.tile_pool(name="awts", bufs=1) as awts,
                tc.tile_pool(name="asb", bufs=2) as asb,
                tc.tile_pool(name="astate", bufs=1) as astate,
                tc.tile_pool(name="apsA", bufs=5, space="PSUM") as apsA,
                tc.tile_pool(name="apsS", bufs=3, space="PSUM") as apsS,
            ):
                tri_sb = acst.tile([128, 128], BF16)
                nc.sync.dma_start(tri_sb[:], TRI[:])
                oblend_sb = acst.tile([128, 128], BF16)
                nc.sync.dma_start(oblend_sb[:], OBLEND[:])
                ok1_sb = acst.tile([1, 128], BF16)
                nc.sync.dma_start(ok1_sb[:], OK1[:])
                ident_sb = acst.tile([128, 128], BF16)
                nc.sync.dma_start(ident_sb[:], IDENT[:])

                wq_sb = awts.tile([128, ECH, c.DPC], BF16)
                nc.sync.dma_start(wq_sb[:], WQ[:].rearrange("(c p) d -> p c d", p=128))
                wk_sb = awts.tile([128, ECH, c.DPC], BF16)
                nc.sync.dma_start(wk_sb[:], WK[:].rearrange("(c p) d -> p c d", p=128))
                wv_sb = awts.tile([128, ECH, c.DPC], BF16)
                nc.sync.dma_start(wv_sb[:], WV[:].rearrange("(c p) d -> p c d", p=128))
                bqkv_sb = {}
                for nme in ("bq", "bk", "bv"):
                    if flags["bqkv"]:
                        t = awts.tile([128, c.DPC], FP32, name=f"{nme}_sb")
                        nc.sync.dma_start(t[:], bio[nme][:])
                        bqkv_sb[nme] = t

                # per-(batch, head) scan state, all batches live (seg-outer)
                sts = {}
                stbs = {}
                if "A" in parts:
                    for b in range(c.B):
                        for h in range(HPC):
                            sts[(b, h)] = astate.tile(
                                [128, DH + 1], FP32, name=f"st_{b}_{h}",
                                tag=f"st{b}{h}", bufs=1)
                            stbs[(b, h)] = astate.tile(
                                [128, DH + 1], BF16, name=f"stb_{b}_{h}",
                                tag=f"stb{b}{h}", bufs=1)

                for seg in range(c.NSEG if "A" in parts else 0):
                    for b in range(c.B):
                        t0 = b * c.S + seg * SEG
                        xt = asb.tile([128, ECH, SEG], BF16, tag="xt")
                        nc.sync.dma_start(
                            xt[:],
                            XT[:].rearrange("(c p) t -> p c t", p=128)[:, :, t0:t0 + SEG])

                        qT = asb.tile([128, HPC * SEG], BF16, tag="qT")
                        kT = asb.tile([128, HPC * SEG], BF16, tag="kT")
                        sqT = asb.tile([128, HPC * SEG], BF16, tag="sqT")
                        skT = asb.tile([128, HPC * SEG], BF16, tag="skT")
                        for (W_sb, dst, elu_dst, bkey) in (
                                (wq_sb, qT, sqT, "bq"), (wk_sb, kT, skT, "bk")):
                            for dt in range(HPC):
                                ps = apsA.tile([128, SEG], FP32, tag="pA", name="ps_qk")
                                for ec in range(ECH):
                                    nc.tensor.matmul(
                                        ps[:], W_sb[:, ec, dt * 128:(dt + 1) * 128],
                                        xt[:, ec, :],
                                        start=(ec == 0), stop=(ec == ECH - 1))
                                sl = slice(dt * SEG, (dt + 1) * SEG)
                                if flags["bqkv"]:
                                    nc.vector.tensor_scalar_add(
                                        ps[:], ps[:],
                                        bqkv_sb[bkey][:, dt * 128:dt * 128 + 1])
                                nc.scalar.copy(dst[:, sl], ps[:])
                                ex = asb.tile([128, SEG], BF16, tag="elu_ex")
                                nc.scalar.activation(ex[:], ps[:], AF.Exp)
                                rl = asb.tile([128, SEG], BF16, tag="elu_rl")
                                nc.vector.tensor_scalar_max(rl[:], ps[:], 0.0)
                                nc.vector.scalar_tensor_tensor(
                                    elu_dst[:, sl], ex[:], 1.0, rl[:],
                                    op0=ALU.min, op1=ALU.add)
                        v_tiles = []
                        for tt in range(NJC):
                            ps = apsA.tile([128, c.DPC], FP32, tag="pA", name="ps_v")
                            for ec in range(ECH):
                                nc.tensor.matmul(
                                    ps[:], xt[:, ec, tt * 128:(tt + 1) * 128],
                                    wv_sb[:, ec, :],
                                    start=(ec == 0), stop=(ec == ECH - 1))
                            if flags["bqkv"]:
                                vb = asb.tile([128, c.DPC], FP32, tag="vb")
                                nc.vector.tensor_add(vb[:], ps[:], bqkv_sb["bv"][:])
                                vt = asb.tile([128, c.DPC], BF16, tag="vnat", bufs=2 * NJC)
                                nc.scalar.copy(vt[:], vb[:])
                            else:
                                vt = asb.tile([128, c.DPC], BF16, tag="vnat", bufs=2 * NJC)
                                nc.scalar.copy(vt[:], ps[:])
                            v_tiles.append(vt)

                        for h in range(HPC):
                            hs = slice(h * SEG, (h + 1) * SEG)
                            st, stb = sts[(b, h)], stbs[(b, h)]
                            if seg > 0:
                                nc.vector.tensor_copy(stb[:], st[:])
                            # scores -> exp -> mask; causal: columns < jc*128
                            # are fully masked and never computed
                            expP = asb.tile([128, NJC, SEG], BF16, tag="expP")
                            for jc in range(NJC):
                                w = SEG - jc * 128
                                ps = apsA.tile([128, SEG], FP32, tag="pA", name="ps_s")
                                nc.tensor.matmul(
                                    ps[:, 0:w],
                                    kT[:, h * SEG + jc * 128: h * SEG + (jc + 1) * 128],
                                    qT[:, h * SEG + jc * 128: (h + 1) * SEG],
                                    start=True, stop=True)
                                nc.scalar.activation(
                                    expP[:, jc, jc * 128:SEG], ps[:, 0:w], AF.Exp,
                                    scale=scale)
                                nc.vector.tensor_mul(
                                    expP[:, jc, jc * 128:(jc + 1) * 128],
                                    expP[:, jc, jc * 128:(jc + 1) * 128], tri_sb[:])
                            # srow (scaled by 1/(1-g)) broadcast
                            ps_r = apsA.tile([128, SEG], FP32, tag="pA", name="ps_r")
                            for jc in range(NJC):
                                nc.tensor.matmul(
                                    ps_r[:, jc * 128:SEG], oblend_sb[:],
                                    expP[:, jc, jc * 128:SEG],
                                    start=(jc == 0), stop=(jc == NJC - 1))
                            r1 = asb.tile([128, SEG], FP32, tag="r1")
                            nc.vector.reciprocal_approx_fast(r1[:], ps_r[:])
                            # A_dot unnormalized
                            ps_a = apsA.tile([128, SEG], FP32, tag="pA", name="ps_a")
                            for jc in range(NJC):
                                nc.tensor.matmul(
                                    ps_a[:, jc * 128:SEG],
                                    v_tiles[jc][:, h * DH:(h + 1) * DH],
                                    expP[:, jc, jc * 128:SEG],
                                    start=(jc == 0), stop=(jc == NJC - 1))
                            outT = asb.tile([128, SEG], BF16, tag="outT")
                            if seg == 0:
                                nc.vector.tensor_mul(outT[:], ps_a[:], r1[:])
                            else:
                                # A_mem unnormalized + den_q
                                ps_m = apsA.tile([128, SEG], FP32, tag="pA", name="ps_m")
                                nc.tensor.matmul(ps_m[:], stb[:, 0:DH], sqT[:, hs],
                                                 start=True, stop=True)
                                ps_d = apsS.tile([1, SEG], FP32, tag="pS", name="ps_d")
                                nc.tensor.matmul(ps_d[:], stb[:, DH:DH + 1], sqT[:, hs],
                                                 start=True, stop=True)
                                dq = asb.tile([1, SEG], BF16, tag="dq")
                                nc.vector.tensor_scalar(
                                    dq[:], ps_d[:], 1.0 / g, 1e-6 / g,
                                    op0=ALU.mult, op1=ALU.add)
                                ps_db = apsA.tile([128, SEG], FP32, tag="pA", name="ps_db")
                                nc.tensor.matmul(ps_db[:], ok1_sb[:], dq[:],
                                                 start=True, stop=True)
                                r2 = asb.tile([128, SEG], FP32, tag="r2")
                                nc.vector.reciprocal_approx_fast(r2[:], ps_db[:])
                                tmb = asb.tile([128, SEG], FP32, tag="tmb")
                                nc.vector.tensor_mul(tmb[:], ps_m[:], r2[:])
                                tab = asb.tile([128, SEG], FP32, tag="tab")
                                nc.vector.tensor_mul(tab[:], ps_a[:], r1[:])
                                nc.vector.tensor_add(outT[:], tmb[:], tab[:])
                            # v_delta (negated) + ones col, then state update
                            ps_u = apsS.tile([128, DH + 1], FP32, tag="pS", name="ps_u")
                            for jc in range(NJC):
                                vd = asb.tile([128, DH + 1], BF16, tag="vd")
                                vch = v_tiles[jc][:, h * DH:(h + 1) * DH]
                                if seg == 0:
                                    nc.vector.tensor_scalar_mul(vd[:, 0:DH], vch, -1.0)
                                else:
                                    ps_t = apsS.tile([128, DH + 1], FP32, tag="pS",
                                                     name="ps_t")
                                    nc.tensor.matmul(
                                        ps_t[:],
                                        skT[:, h * SEG + jc * 128:h * SEG + (jc + 1) * 128],
                                        stb[:], start=True, stop=True)
                                    dkr = asb.tile([128, 1], FP32, tag="dkr")
                                    nc.vector.tensor_scalar_add(
                                        dkr[:], ps_t[:, DH:DH + 1], 1e-6)
                                    rk = asb.tile([128, 1], FP32, tag="rk")
                                    nc.vector.reciprocal_approx_fast(rk[:], dkr[:])
                                    nc.vector.scalar_tensor_tensor(
                                        vd[:, 0:DH], ps_t[:, 0:DH], rk[:], vch,
                                        op0=ALU.mult, op1=ALU.subtract)
                                nc.vector.memset(vd[:, DH:DH + 1], -1.0)
                                # sk_nat via PE transpose
                                ps_tr = apsS.tile([128, 128], BF16, tag="pS", name="ps_tr")
                                nc.tensor.transpose(
                                    ps_tr[:],
                                    skT[:, h * SEG + jc * 128:h * SEG + (jc + 1) * 128],
                                    ident_sb[:])
                                skn = asb.tile([128, 128], BF16, tag="skn")
                                nc.scalar.copy(skn[:], ps_tr[:])
                                nc.tensor.matmul(ps_u[:], skn[:], vd[:],
                                                 start=(jc == 0), stop=(jc == NJC - 1))
                            if seg == 0:
                                nc.vector.tensor_scalar_mul(st[:], ps_u[:], -1.0)
                            else:
                                nc.vector.tensor_sub(st[:], st[:], ps_u[:])
                            # stage attention output for A2A: quarter = seg%4
                            idx = b * c.NSEG + seg
                            nc.sync.dma_start(
                                a2a_in[seg % NQ][idx // NQ,
                                                 h * DH:(h + 1) * DH, :],
                                outT[:])
                    # quarter q's last producer is seg == q + NQ
                    if a2a and "A" in parts and seg >= NQ:
                        q = seg - NQ
                        nc.gpsimd.collective_compute(
                            "AllToAll", ALU.bypass,
                            replica_groups=[list(range(N_CORES))],
                            ins=[a2a_in[q].opt()], outs=[a2a_out[q].opt()])

            # ======================= PHASE B =======================
            HH = HDCH // 2   # half of the e-chunks
            EH = ECH // 2
            W2C = FCH // 8   # w2 streamed in eighth-chunks
            with (
                tc.tile_pool(name="bffn", bufs=1) as bf,
                tc.tile_pool(name="bpsB", bufs=4, space="PSUM") as bpsB,
                tc.tile_pool(name="bpsS", bufs=2, space="PSUM") as bpsS,
            ):
                for tb in range(NTB if "B" in parts else 0):
                    ts0 = tb * TBLK
                    aTh = []
                    for hf in range(2):
                        at = b1.tile([128, HH, TBLK], BF16, tag="aTh", bufs=3)
                        nc.sync.dma_start(
                            at[:],
                            a2a_out[tb][:].rearrange(
                                "s (q p) t -> p (s q) t", p=128)[:, hf * HH:(hf + 1) * HH, :])
                        aTh.append(at)
                    # ---- O-projection + rmsnorm1 stats ----
                    sq_acc = bf.tile([128, TBLK], FP32, tag="sqacc", bufs=1)
                    y_bf = []
                    for et in range(HDCH):
                        ps = bpsB.tile([128, TBLK], FP32, tag="pB", name="ps_y")
                        for hf in range(2):
                            wo_t = b1.tile([128, HH, 128], BF16, tag="wo", bufs=2)
                            nc.sync.dma_start(
                                wo_t[:],
                                WO[hf * (E // 2):(hf + 1) * (E // 2),
                                   et * 128:(et + 1) * 128].rearrange(
                                    "(q p) e -> p q e", p=128))
                            for hc in range(HH):
                                gc = hf * HH + hc
                                nc.tensor.matmul(ps[:], wo_t[:, hc, :],
                                                 aTh[hf][:, hc, :],
                                                 start=(gc == 0), stop=(gc == HDCH - 1))
                        if flags["bo"]:
                            nc.vector.tensor_scalar_add(
                                ps[:], ps[:], bias_sb["bo"][et, :].reshape([128, 1]))
                        yb = bf.tile([128, TBLK], BF16, tag="ybf", bufs=HDCH + 1)
                        nc.scalar.copy(yb[:], ps[:])
                        y_bf.append(yb)
                        ysq = bf.tile([128, TBLK], BF16, tag="ysq", bufs=2)
                        nc.scalar.activation(ysq[:], ps[:], AF.Square)
                        if et == 0:
                            nc.vector.tensor_copy(sq_acc[:], ysq[:])
                        else:
                            nc.vector.tensor_add(sq_acc[:], sq_acc[:], ysq[:])
                    sq_bf = bf.tile([128, TBLK], BF16, tag="sqbf", bufs=1)
                    nc.scalar.copy(sq_bf[:], sq_acc[:])
                    ps_ss = bpsS.tile([128, TBLK], FP32, tag="pSt", name="ps_ss")
                    nc.tensor.matmul(ps_ss[:], ones_sb[:], sq_bf[:],
                                     start=True, stop=True)
                    sqs = bf.tile([128, TBLK], FP32, tag="sqs", bufs=1)
                    nc.scalar.activation(sqs[:], ps_ss[:], AF.Sqrt,
                                         scale=1.0 / E, bias=eps_sb[:])
                    rstd = bf.tile([128, TBLK], FP32, tag="rstd", bufs=1)
                    nc.vector.reciprocal_approx_fast(rstd[:], sqs[:])
                    # ---- x1 = x + rmsnorm1(y) (fused) ----
                    x1_t = []
                    for et in range(HDCH):
                        xs = bf.tile([128, TBLK], BF16, tag="xs", bufs=2)
                        nc.sync.dma_start(xs[:], XSL[et * 128:(et + 1) * 128,
                                                     ts0:ts0 + TBLK])
                        x1 = bf.tile([128, TBLK], BF16, tag="x1b", bufs=HDCH + 1)
                        n1 = bf.tile([128, TBLK], BF16, tag="n1t", bufs=2)
                        nc.vector.tensor_mul(n1[:], y_bf[et][:], rstd[:])
                        if flags["aw"]:
                            nc.vector.tensor_scalar_mul(
                                n1[:], n1[:], bias_sb["aw"][et, :].reshape([128, 1]))
                        nc.vector.tensor_add(x1[:], xs[:], n1[:])
                        x1_t.append(x1)
                    # ---- FFN up + gate ----
                    h_t = []
                    for ft in range(FCH):
                        ps1 = bpsB.tile([128, TBLK], FP32, tag="pB", name="ps_h1")
                        ps3 = bpsB.tile([128, TBLK], FP32, tag="pB", name="ps_h3")
                        for eh in range(2):
                            w1_t = bf.tile([128, EH, 128], BF16, tag="w1", bufs=2)
                            nc.sync.dma_start(
                                w1_t[:],
                                W1[eh * (E // 2):(eh + 1) * (E // 2),
                                   ft * 128:(ft + 1) * 128].rearrange(
                                    "(c p) f -> p c f", p=128))
                            for ec in range(EH):
                                gc = eh * EH + ec
                                nc.tensor.matmul(ps1[:], w1_t[:, ec, :], x1_t[gc][:],
                                                 start=(gc == 0), stop=(gc == ECH - 1))
                        for eh in range(2):
                            w3_t = bf.tile([128, EH, 128], BF16, tag="w3", bufs=2)
                            nc.sync.dma_start(
                                w3_t[:],
                                W3[eh * (E // 2):(eh + 1) * (E // 2),
                                   ft * 128:(ft + 1) * 128].rearrange(
                                    "(c p) f -> p c f", p=128))
                            for ec in range(EH):
                                gc = eh * EH + ec
                                nc.tensor.matmul(ps3[:], w3_t[:, ec, :], x1_t[gc][:],
                                                 start=(gc == 0), stop=(gc == ECH - 1))
                        g1 = bf.tile([128, TBLK], BF16, tag="g1", bufs=2)
                        if flags["b13"]:
                            nc.scalar.activation(
                                g1[:], ps1[:], AF.Gelu,
                                bias=bias_sb["b1"][ft, :].reshape([128, 1]))
                        else:
                            nc.scalar.activation(g1[:], ps1[:], AF.Gelu)
                        ht = bf.tile([128, TBLK], BF16, tag="h", bufs=FCH)
                        if flags["b13"]:
                            nc.vector.scalar_tensor_tensor(
                                ht[:], ps3[:], bias_sb["b3"][ft, :].reshape([128, 1]),
                                g1[:], op0=ALU.add, op1=ALU.mult)
                        else:
                            nc.vector.tensor_mul(ht[:], ps3[:], g1[:])
                        h_t.append(ht)
                    # ---- FFN down + rmsnorm2 + out ----
                    sq_acc2 = bf.tile([128, TBLK], FP32, tag="sqacc2", bufs=1)
                    y2_bf = []
                    for et in range(HDCH):
                        ps = bpsB.tile([128, TBLK], FP32, tag="pB", name="ps_y2")
                        for w8 in range(8):
                            w2_t = bf.tile([128, W2C, 128], BF16, tag="w2", bufs=2)
                            nc.sync.dma_start(
                                w2_t[:],
                                W2[w8 * (c.F // 8):(w8 + 1) * (c.F // 8),
                                   et * 128:(et + 1) * 128].rearrange(
                                    "(q p) e -> p q e", p=128))
                            for fq in range(W2C):
                                fc = w8 * W2C + fq
                                nc.tensor.matmul(
                                    ps[:], w2_t[:, fq, :], h_t[fc][:],
                                    start=(fc == 0), stop=(fc == FCH - 1))
                        if flags["b2"]:
                            nc.vector.tensor_scalar_add(
                                ps[:], ps[:], bias_sb["b2"][et, :].reshape([128, 1]))
                        y2 = bf.tile([128, TBLK], BF16, tag="y2bf", bufs=HDCH + 1)
                        nc.scalar.copy(y2[:], ps[:])
                        y2_bf.append(y2)
                        y2sq = bf.tile([128, TBLK], BF16, tag="y2sq", bufs=2)
                        nc.scalar.activation(y2sq[:], ps[:], AF.Square)
                        if et == 0:
                            nc.vector.tensor_copy(sq_acc2[:], y2sq[:])
                        else:
                            nc.vector.tensor_add(sq_acc2[:], sq_acc2[:], y2sq[:])
                    sq2_bf = bf.tile([128, TBLK], BF16, tag="sq2bf", bufs=1)
                    nc.scalar.copy(sq2_bf[:], sq_acc2[:])
                    ps_ss2 = bpsS.tile([128, TBLK], FP32, tag="pSt", name="ps_ss2")
                    nc.tensor.matmul(ps_ss2[:], ones_sb[:], sq2_bf[:],
                                     start=True, stop=True)
                    sqs2 = bf.tile([128, TBLK], FP32, tag="sqs2", bufs=1)
                    nc.scalar.activation(sqs2[:], ps_ss2[:], AF.Sqrt,
                                         scale=1.0 / E, bias=eps_sb[:])
                    rstd2 = bf.tile([128, TBLK], FP32, tag="rstd2", bufs=1)
                    nc.vector.reciprocal_approx_fast(rstd2[:], sqs2[:])
                    for et in range(HDCH):
                        of = bf.tile([128, TBLK], FP32, tag="of", bufs=2)
                        n2 = bf.tile([128, TBLK], BF16, tag="n2", bufs=2)
                        nc.vector.tensor_mul(n2[:], y2_bf[et][:], rstd2[:])
                        if flags["fw"]:
                            nc.vector.tensor_scalar_mul(
                                n2[:], n2[:], bias_sb["fw"][et, :].reshape([128, 1]))
                        nc.vector.tensor_add(of[:], x1_t[et][:], n2[:])
                        nc.sync.dma_start(OUT[et * 128:(et + 1) * 128, ts0:ts0 + TBLK],
                                          of[:])
            b1_cm.__exit__(None, None, None)
    return tc


def _prep_inputs(inputs, cfg):
    """Host-side: slice/transpose/cast per core. Returns (in_maps, g, flags)."""
    c = cfg
    f32 = np.float32
    x = np.ascontiguousarray(np.asarray(inputs["x"], f32).reshape(c.BS, c.E))
    beta = float(np.asarray(inputs["beta"]).reshape(-1)[0])
    g = 1.0 / (1.0 + math.exp(-beta))
    g = min(max(g, 1e-6), 1.0 - 1e-6)

    Wq = np.asarray(inputs["Wq"], f32)
    Wk = np.asarray(inputs["Wk"], f32)
    Wv = np.asarray(inputs["Wv"], f32)
    Wo = np.asarray(inputs["Wo"], f32)
    W1 = np.asarray(inputs["W1"], f32)
    W2 = np.asarray(inputs["W2"], f32)
    W3 = np.asarray(inputs["W3"], f32)
    aw = np.asarray(inputs["attn_norm_w"], f32)
    fw = np.asarray(inputs["ffn_norm_w"], f32)
    bq = np.asarray(inputs["bq"], f32)
    bk = np.asarray(inputs["bk"], f32)
    bv = np.asarray(inputs["bv"], f32)
    bo = np.asarray(inputs["bo"], f32)
    b1 = np.asarray(inputs["b1"], f32)
    b2 = np.asarray(inputs["b2"], f32)
    b3 = np.asarray(inputs["b3"], f32)

    flags = {
        "bqkv": bool(np.any(bq) or np.any(bk) or np.any(bv)),
        "bo": bool(np.any(bo)),
        "b13": bool(np.any(b1) or np.any(b3)),
        "b2": bool(np.any(b2)),
        "aw": not bool(np.all(aw == 1.0)),
        "fw": not bool(np.all(fw == 1.0)),
    }

    xT = np.ascontiguousarray(x.T).astype(NPBF16)
    woT = np.ascontiguousarray(Wo.T).astype(NPBF16)
    w1T = np.ascontiguousarray(W1.T).astype(NPBF16)
    w3T = np.ascontiguousarray(W3.T).astype(NPBF16)
    w2T = np.ascontiguousarray(W2.T).astype(NPBF16)

    in_maps = []
    for core in range(N_CORES):
        ds = slice(core * c.DPC, (core + 1) * c.DPC)
        tok = slice(core * c.TSL, (core + 1) * c.TSL)
        m = {
            "xT": xT,
            "xslb": np.ascontiguousarray(x[tok].T).astype(NPBF16),
            "wq": np.ascontiguousarray(Wq[ds].T).astype(NPBF16),
            "wk": np.ascontiguousarray(Wk[ds].T).astype(NPBF16),
            "wv": np.ascontiguousarray(Wv[ds].T).astype(NPBF16),
            "wo": woT, "w1": w1T, "w3": w3T, "w2": w2T,
        }
        if flags["bqkv"]:
            m["bq"] = np.tile(bq[ds][None, :], (128, 1)).astype(f32)
            m["bk"] = np.tile(bk[ds][None, :], (128, 1)).astype(f32)
            m["bv"] = np.tile(bv[ds][None, :], (128, 1)).astype(f32)
        if flags["bo"]:
            m["bo"] = bo.reshape(c.E // 128, 128).astype(f32)
        if flags["b13"]:
            m["b1"] = b1.reshape(c.FCH, 128).astype(f32)
            m["b3"] = b3.reshape(c.FCH, 128).astype(f32)
        if flags["b2"]:
            m["b2"] = b2.reshape(c.E // 128, 128).astype(f32)
        if flags["aw"]:
            m["aw"] = aw.reshape(c.E // 128, 128).astype(f32)
        if flags["fw"]:
            m["fw"] = fw.reshape(c.E // 128, 128).astype(f32)
        in_maps.append(m)
    return in_maps, g, flags


def run(inputs, cfg, trace=False):
    in_maps, g, flags = _prep_inputs(inputs, cfg)
    nc = bacc.Bacc("TRN2", target_bir_lowering=False, debug=False,
                   num_devices=N_CORES)
    build(nc, cfg, g, flags)
    nc.compile()
    res = run_bass_kernel_spmd(nc, in_maps, list(range(N_CORES)), trace=trace)
    out = np.empty((cfg.BS, cfg.E), np.float32)
    for core in range(N_CORES):
        out[core * cfg.TSL:(core + 1) * cfg.TSL] = res.results[core]["outT"].T
    return out.reshape(cfg.B, cfg.S, cfg.E), res


def kernel(**inputs):
    out, _ = run(inputs, FULL)
    return out


# revision 11
# speedup vs baseline: 1.0850x; 1.0715x over previous
"""Infini-attention decoder block on 8 Trainium2 NeuronCores.

Strategy (fully transposed activation layout: features on SBUF partitions,
tokens on the free dim):
  Phase A (head-parallel): core c owns heads {2c, 2c+1}. Streams x^T once,
    computes q^T/k^T (transposed) + v (natural) projections per (segment,
    batch) — seg-outer so the compressive-memory scan states for all 4
    batches advance together — with all the linear-attention algebra
    expressed as PE matmuls. Causally-masked column blocks of the local
    softmax are never computed (partial-N matmuls).
  AllToAll (x4): the attention output is resharded head->token in four
    token-quarter collectives; quarter q's last producer is segment q+4,
    so the collectives overlap the tail of phase A and phase B block 0
    starts with its data already resident.
  Phase B (token-parallel): core c owns 2048 tokens. O-projection, fused
    rmsnorm+residual, SwiGLU FFN with streamed weights, final norm+residual.
    Persistent tile pools let consecutive 512-token blocks overlap.
Host pre-transposes/casts inputs per core and reassembles the output.
"""
import math

import numpy as np
import ml_dtypes

import concourse.bass as bass
import concourse.mybir as mybir
import concourse.tile as tile
from concourse import bacc
from concourse.bass_utils import run_bass_kernel_spmd

AF = mybir.ActivationFunctionType
ALU = mybir.AluOpType
BF16 = mybir.dt.bfloat16
FP32 = mybir.dt.float32
NPBF16 = ml_dtypes.bfloat16

N_CORES = 8


class Cfg:
    def __init__(self, B, S, E, H, DH, NSEG):
        self.B, self.S, self.E, self.H, self.DH, self.NSEG = B, S, E, H, DH, NSEG
        self.SEG = S // NSEG
        self.BS = B * S
        self.ECH = E // 128           # e-chunks
        self.HPC = H // N_CORES       # heads per core
        self.DPC = self.HPC * DH      # head dims per core
        self.NJC = self.SEG // 128    # 128-row chunks per segment
        self.TSL = self.BS // N_CORES  # tokens per core in phase B
        self.TBLK = min(512, self.TSL)
        self.NTB = self.TSL // self.TBLK
        self.F = 4 * E
        self.FCH = self.F // 128
        assert self.SEG % 128 == 0 and self.TSL % self.SEG == 0
        assert self.DPC % 128 == 0 and E % 128 == 0


FULL = Cfg(B=4, S=4096, E=2048, H=16, DH=128, NSEG=8)


def build(nc, cfg, g, flags, a2a=True, parts="AB"):
    """Emit the SPMD program. g = sigmoid(beta). flags: which bias/normw paths
    are active (compile-time; values are baked per-call)."""
    c = cfg
    E, DH, SEG, ECH, HPC, NJC, TSL, TBLK, NTB, FCH = (
        c.E, c.DH, c.SEG, c.ECH, c.HPC, c.NJC, c.TSL, c.TBLK, c.NTB, c.FCH)
    HDCH = E // 128
    scale = 1.0 / math.sqrt(DH)
    NQ = NTB  # token quarters == phase-B blocks
    assert SEG == TBLK and c.NSEG == 2 * NQ

    # ---------------- I/O ----------------
    XT = nc.dram_tensor("xT", [E, c.BS], BF16, kind="ExternalInput")
    XSL = nc.dram_tensor("xslb", [E, TSL], BF16, kind="ExternalInput")
    WQ = nc.dram_tensor("wq", [E, c.DPC], BF16, kind="ExternalInput")
    WK = nc.dram_tensor("wk", [E, c.DPC], BF16, kind="ExternalInput")
    WV = nc.dram_tensor("wv", [E, c.DPC], BF16, kind="ExternalInput")
    WO = nc.dram_tensor("wo", [E, E], BF16, kind="ExternalInput")
    W1 = nc.dram_tensor("w1", [E, c.F], BF16, kind="ExternalInput")
    W3 = nc.dram_tensor("w3", [E, c.F], BF16, kind="ExternalInput")
    W2 = nc.dram_tensor("w2", [c.F, E], BF16, kind="ExternalInput")
    OUT = nc.dram_tensor("outT", [E, TSL], FP32, kind="ExternalOutput")
    bio = {}
    if flags["bqkv"]:
        for nme in ("bq", "bk", "bv"):
            bio[nme] = nc.dram_tensor(nme, [128, c.DPC], FP32, kind="ExternalInput")
    if flags["bo"]:
        bio["bo"] = nc.dram_tensor("bo", [HDCH, 128], FP32, kind="ExternalInput")
    if flags["b13"]:
        bio["b1"] = nc.dram_tensor("b1", [FCH, 128], FP32, kind="ExternalInput")
        bio["b3"] = nc.dram_tensor("b3", [FCH, 128], FP32, kind="ExternalInput")
    if flags["b2"]:
        bio["b2"] = nc.dram_tensor("b2", [HDCH, 128], FP32, kind="ExternalInput")
    if flags["aw"]:
        bio["aw"] = nc.dram_tensor("aw", [HDCH, 128], FP32, kind="ExternalInput")
    if flags["fw"]:
        bio["fw"] = nc.dram_tensor("fw", [HDCH, 128], FP32, kind="ExternalInput")

    # inline constants
    tri_np = np.triu(np.ones((128, 128), np.float32)).astype(NPBF16)
    TRI = nc.inline_tensor(tri_np, name="tri")
    ONES = nc.inline_tensor(np.ones((128, 128), NPBF16), name="ones_c")
    OBLEND = nc.inline_tensor(
        np.full((128, 128), 1.0 / (1.0 - g), np.float32).astype(NPBF16), name="oblend")
    OK1 = nc.inline_tensor(np.ones((1, 128), NPBF16), name="ok1")
    IDENT = nc.inline_tensor(np.eye(128, dtype=np.float32).astype(NPBF16), name="ident")

    tc = tile.TileContext(nc, num_cores=N_CORES)
    with tc:
        with tc.tile_pool(name="dramp", bufs=1, space="DRAM") as dramp:
            a2a_in = [dramp.tile([N_CORES, c.DPC, TBLK], BF16, name=f"a2ai{q}")
                      for q in range(NQ)]
            if a2a:
                a2a_out = [dramp.tile([N_CORES, c.DPC, TBLK], BF16,
                                      name=f"a2ao{q}") for q in range(NQ)]
            else:
                a2a_out = a2a_in

            # Persistent pool for phase-B stage 1 (O-proj / norm1): opened
            # before phase A so its tiles do not alias phase-A space and
            # block 0 can overlap phase A's tail.
            b1_cm = tc.tile_pool(name="bpb1", bufs=1)
            b1 = b1_cm.__enter__()
            ones_sb = b1.tile([128, 128], BF16, name="ones_sb")
            nc.scalar.dma_start(ones_sb[:], ONES[:])
            eps_sb = b1.tile([128, 1], FP32, name="eps_sb")
            nc.vector.memset(eps_sb[:], 1e-5)
            bias_sb = {}
            for nme in ("bo", "b2", "aw", "fw", "b1", "b3"):
                if nme in bio:
                    t = b1.tile(list(bio[nme].shape), FP32, name=f"{nme}_sb")
                    nc.sync.dma_start(t[:], bio[nme][:])
                    bias_sb[nme] = t

            aTh_pre = {}
            # ======================= PHASE A =======================
            with (
                tc.tile_pool(name="acst", bufs=1) as acst,
                tc.tile_pool(name="awts", bufs=1) as awts,
                tc.tile_pool(name="asb", bufs=2) as asb,
                tc.tile_pool(name="astate", bufs=1) as astate,
                tc.tile_pool(name="apsA", bufs=5, space="PSUM") as apsA,
                tc.tile_pool(name="apsS", bufs=3, space="PSUM") as apsS,
            ):
                tri_sb = acst.tile([128, 128], BF16)
                nc.scalar.dma_start(tri_sb[:], TRI[:])
                oblend_sb = acst.tile([128, 128], BF16)
                nc.scalar.dma_start(oblend_sb[:], OBLEND[:])
                ok1_sb = acst.tile([1, 128], BF16)
                nc.scalar.dma_start(ok1_sb[:], OK1[:])
                ident_sb = acst.tile([128, 128], BF16)
                nc.scalar.dma_start(ident_sb[:], IDENT[:])

                wq_sb = awts.tile([128, ECH, c.DPC], BF16)
                nc.sync.dma_start(wq_sb[:], WQ[:].rearrange("(c p) d -> p c d", p=128))
                wk_sb = awts.tile([128, ECH, c.DPC], BF16)
                nc.sync.dma_start(wk_sb[:], WK[:].rearrange("(c p) d -> p c d", p=128))
                wv_sb = awts.tile([128, ECH, c.DPC], BF16)
                nc.gpsimd.dma_start(wv_sb[:], WV[:].rearrange("(c p) d -> p c d", p=128))
                bqkv_sb = {}
                for nme in ("bq", "bk", "bv"):
                    if flags["bqkv"]:
                        t = awts.tile([128, c.DPC], FP32, name=f"{nme}_sb")
                        nc.sync.dma_start(t[:], bio[nme][:])
                        bqkv_sb[nme] = t

                # per-(batch, head) scan state, all batches live (seg-outer)
                sts = {}
                stbs = {}
                if "A" in parts:
                    for b in range(c.B):
                        for h in range(HPC):
                            sts[(b, h)] = astate.tile(
                                [128, DH + 1], FP32, name=f"st_{b}_{h}",
                                tag=f"st{b}{h}", bufs=1)
                            stbs[(b, h)] = astate.tile(
                                [128, DH + 1], BF16, name=f"stb_{b}_{h}",
                                tag=f"stb{b}{h}", bufs=1)

                for seg in range(c.NSEG if "A" in parts else 0):
                    for b in range(c.B):
                        t0 = b * c.S + seg * SEG
                        xt = asb.tile([128, ECH, SEG], BF16, tag="xt")
                        nc.sync.dma_start(
                            xt[:],
                            XT[:].rearrange("(c p) t -> p c t", p=128)[:, :, t0:t0 + SEG])

                        qT = asb.tile([128, HPC * SEG], BF16, tag="qT")
                        kT = asb.tile([128, HPC * SEG], BF16, tag="kT")
                        sqT = asb.tile([128, HPC * SEG], BF16, tag="sqT")
                        skT = asb.tile([128, HPC * SEG], BF16, tag="skT")
                        for (W_sb, dst, elu_dst, bkey) in (
                                (wq_sb, qT, sqT, "bq"), (wk_sb, kT, skT, "bk")):
                            for dt in range(HPC):
                                ps = apsA.tile([128, SEG], FP32, tag="pA", name="ps_qk")
                                for ec in range(ECH):
                                    nc.tensor.matmul(
                                        ps[:], W_sb[:, ec, dt * 128:(dt + 1) * 128],
                                        xt[:, ec, :],
                                        start=(ec == 0), stop=(ec == ECH - 1))
                                sl = slice(dt * SEG, (dt + 1) * SEG)
                                if flags["bqkv"]:
                                    nc.vector.tensor_scalar_add(
                                        ps[:], ps[:],
                                        bqkv_sb[bkey][:, dt * 128:dt * 128 + 1])
                                nc.scalar.copy(dst[:, sl], ps[:])
                                ex = asb.tile([128, SEG], BF16, tag="elu_ex")
                                nc.scalar.activation(ex[:], ps[:], AF.Exp)
                                rl = asb.tile([128, SEG], BF16, tag="elu_rl")
                                nc.vector.tensor_scalar_max(rl[:], ps[:], 0.0)
                                nc.vector.scalar_tensor_tensor(
                                    elu_dst[:, sl], ex[:], 1.0, rl[:],
                                    op0=ALU.min, op1=ALU.add)
                        v_tiles = []
                        for tt in range(NJC):
                            ps = apsA.tile([128, c.DPC], FP32, tag="pA", name="ps_v")
                            for ec in range(ECH):
                                nc.tensor.matmul(
                                    ps[:], xt[:, ec, tt * 128:(tt + 1) * 128],
                                    wv_sb[:, ec, :],
                                    start=(ec == 0), stop=(ec == ECH - 1))
                            if flags["bqkv"]:
                                vb = asb.tile([128, c.DPC], FP32, tag="vb")
                                nc.vector.tensor_add(vb[:], ps[:], bqkv_sb["bv"][:])
                                vt = asb.tile([128, c.DPC], BF16, tag="vnat", bufs=2 * NJC)
                                nc.scalar.copy(vt[:], vb[:])
                            else:
                                vt = asb.tile([128, c.DPC], BF16, tag="vnat", bufs=2 * NJC)
                                nc.scalar.copy(vt[:], ps[:])
                            v_tiles.append(vt)

                        for h in range(HPC):
                            hs = slice(h * SEG, (h + 1) * SEG)
                            st, stb = sts[(b, h)], stbs[(b, h)]
                            if seg > 0:
                                nc.vector.tensor_copy(stb[:], st[:])
                            # scores -> exp -> mask; causal: columns < jc*128
                            # are fully masked and never computed
                            expP = asb.tile([128, NJC, SEG], BF16, tag="expP")
                            for jc in range(NJC):
                                w = SEG - jc * 128
                                ps = apsA.tile([128, SEG], FP32, tag="pA", name="ps_s")
                                nc.tensor.matmul(
                                    ps[:, 0:w],
                                    kT[:, h * SEG + jc * 128: h * SEG + (jc + 1) * 128],
                                    qT[:, h * SEG + jc * 128: (h + 1) * SEG],
                                    start=True, stop=True)
                                nc.scalar.activation(
                                    expP[:, jc, jc * 128:SEG], ps[:, 0:w], AF.Exp,
                                    scale=scale)
                                nc.vector.tensor_mul(
                                    expP[:, jc, jc * 128:(jc + 1) * 128],
                                    expP[:, jc, jc * 128:(jc + 1) * 128], tri_sb[:])
                            # srow (scaled by 1/(1-g)) broadcast
                            ps_r = apsA.tile([128, SEG], FP32, tag="pA", name="ps_r")
                            for jc in range(NJC):
                                nc.tensor.matmul(
                                    ps_r[:, jc * 128:SEG], oblend_sb[:],
                                    expP[:, jc, jc * 128:SEG],
                                    start=(jc == 0), stop=(jc == NJC - 1))
                            r1 = asb.tile([128, SEG], FP32, tag="r1")
                            nc.vector.reciprocal_approx_fast(r1[:], ps_r[:])
                            # A_dot unnormalized
                            ps_a = apsA.tile([128, SEG], FP32, tag="pA", name="ps_a")
                            for jc in range(NJC):
                                nc.tensor.matmul(
                                    ps_a[:, jc * 128:SEG],
                                    v_tiles[jc][:, h * DH:(h + 1) * DH],
                                    expP[:, jc, jc * 128:SEG],
                                    start=(jc == 0), stop=(jc == NJC - 1))
                            outT = asb.tile([128, SEG], BF16, tag="outT")
                            if seg == 0:
                                nc.vector.tensor_mul(outT[:], ps_a[:], r1[:])
                            else:
                                # A_mem unnormalized + den_q
                                ps_m = apsA.tile([128, SEG], FP32, tag="pA", name="ps_m")
                                nc.tensor.matmul(ps_m[:], stb[:, 0:DH], sqT[:, hs],
                                                 start=True, stop=True)
                                ps_d = apsS.tile([1, SEG], FP32, tag="pS", name="ps_d")
                                nc.tensor.matmul(ps_d[:], stb[:, DH:DH + 1], sqT[:, hs],
                                                 start=True, stop=True)
                                dq = asb.tile([1, SEG], BF16, tag="dq")
                                nc.vector.tensor_scalar(
                                    dq[:], ps_d[:], 1.0 / g, 1e-6 / g,
                                    op0=ALU.mult, op1=ALU.add)
                                ps_db = apsA.tile([128, SEG], FP32, tag="pA", name="ps_db")
                                nc.tensor.matmul(ps_db[:], ok1_sb[:], dq[:],
                                                 start=True, stop=True)
                                r2 = asb.tile([128, SEG], FP32, tag="r2")
                                nc.vector.reciprocal_approx_fast(r2[:], ps_db[:])
                                tmb = asb.tile([128, SEG], FP32, tag="tmb")
                                nc.vector.tensor_mul(tmb[:], ps_m[:], r2[:])
                                tab = asb.tile([128, SEG], FP32, tag="tab")
                                nc.vector.tensor_mul(tab[:], ps_a[:], r1[:])
                                nc.vector.tensor_add(outT[:], tmb[:], tab[:])
                            # v_delta (negated) + ones col, then state update
                            ps_u = apsS.tile([128, DH + 1], FP32, tag="pS", name="ps_u")
                            for jc in range(NJC):
                                vd = asb.tile([128, DH + 1], BF16, tag="vd")
                                vch = v_tiles[jc][:, h * DH:(h + 1) * DH]
                                if seg == 0:
                                    nc.vector.tensor_scalar_mul(vd[:, 0:DH], vch, -1.0)
                                else:
                                    ps_t = apsS.tile([128, DH + 1], FP32, tag="pS",
                                                     name="ps_t")
                                    nc.tensor.matmul(
                                        ps_t[:],
                                        skT[:, h * SEG + jc * 128:h * SEG + (jc + 1) * 128],
                                        stb[:], start=True, stop=True)
                                    dkr = asb.tile([128, 1], FP32, tag="dkr")
                                    nc.vector.tensor_scalar_add(
                                        dkr[:], ps_t[:, DH:DH + 1], 1e-6)
                                    rk = asb.tile([128, 1], FP32, tag="rk")
                                    nc.vector.reciprocal_approx_fast(rk[:], dkr[:])
                                    nc.vector.scalar_tensor_tensor(
                                        vd[:, 0:DH], ps_t[:, 0:DH], rk[:], vch,
                                        op0=ALU.mult, op1=ALU.subtract)
                                nc.vector.memset(vd[:, DH:DH + 1], -1.0)
                                # sk_nat via PE transpose
                                ps_tr = apsS.tile([128, 128], BF16, tag="pS", name="ps_tr")
                                nc.tensor.transpose(
                                    ps_tr[:],
                                    skT[:, h * SEG + jc * 128:h * SEG + (jc + 1) * 128],
                                    ident_sb[:])
                                skn = asb.tile([128, 128], BF16, tag="skn")
                                nc.scalar.copy(skn[:], ps_tr[:])
                                nc.tensor.matmul(ps_u[:], skn[:], vd[:],
                                                 start=(jc == 0), stop=(jc == NJC - 1))
                            if seg == 0:
                                nc.vector.tensor_scalar_mul(st[:], ps_u[:], -1.0)
                            else:
                                nc.vector.tensor_sub(st[:], st[:], ps_u[:])
                            # stage attention output for A2A: quarter = seg%4
                            idx = b * c.NSEG + seg
                            nc.scalar.dma_start(
                                a2a_in[seg % NQ][idx // NQ,
                                                 h * DH:(h + 1) * DH, :],
                                outT[:])
                    # quarter q's last producer is seg == q + NQ
                    if a2a and "A" in parts and seg >= NQ:
                        q = seg - NQ
                        nc.gpsimd.collective_compute(
                            "AllToAll", ALU.bypass,
                            replica_groups=[list(range(N_CORES))],
                            ins=[a2a_in[q].opt()], outs=[a2a_out[q].opt()])
                        if q == 0 and "B" in parts:
                            # prefetch block 0's O-proj input during phase A
                            pre = []
                            for hf in range(2):
                                at = b1.tile([128, HDCH // 2, TBLK], BF16,
                                             tag="aTh", bufs=3, name="at_pre")
                                nc.gpsimd.dma_start(
                                    at[:],
                                    a2a_out[0][:].rearrange(
                                        "s (q p) t -> p (s q) t", p=128)[
                                        :, hf * (HDCH // 2):(hf + 1) * (HDCH // 2), :])
                                pre.append(at)
                            aTh_pre = {0: pre}

            # ======================= PHASE B =======================
            HH = HDCH // 2   # half of the e-chunks
            EH = ECH // 2
            W2C = FCH // 8   # w2 streamed in eighth-chunks
            with (
                tc.tile_pool(name="bffn", bufs=1) as bf,
                tc.tile_pool(name="bpsB", bufs=6, space="PSUM") as bpsB,
                tc.tile_pool(name="bpsS", bufs=2, space="PSUM") as bpsS,
            ):
                for tb in range(NTB if "B" in parts else 0):
                    ts0 = tb * TBLK
                    if tb in aTh_pre:
                        aTh = aTh_pre.pop(tb)
                    else:
                        aTh = []
                        for hf in range(2):
                            at = b1.tile([128, HH, TBLK], BF16, tag="aTh", bufs=3)
                            nc.sync.dma_start(
                                at[:],
                                a2a_out[tb][:].rearrange(
                                    "s (q p) t -> p (s q) t", p=128)[
                                    :, hf * HH:(hf + 1) * HH, :])
                            aTh.append(at)
                    # ---- O-projection + rmsnorm1 stats ----
                    sq_acc = bf.tile([128, TBLK], FP32, tag="sqacc", bufs=1)
                    y_bf = []
                    for et in range(HDCH):
                        ps = bpsB.tile([128, TBLK], FP32, tag="pB", name="ps_y")
                        wo_t = b1.tile([128, HDCH, 128], BF16, tag="wo", bufs=2)
                        nc.sync.dma_start(
                            wo_t[:],
                            WO[:, et * 128:(et + 1) * 128].rearrange(
                                "(q p) e -> p q e", p=128))
                        for hc in range(HDCH):
                            nc.tensor.matmul(ps[:], wo_t[:, hc, :],
                                             aTh[hc // HH][:, hc % HH, :],
                                             start=(hc == 0), stop=(hc == HDCH - 1))
                        if flags["bo"]:
                            nc.vector.tensor_scalar_add(
                                ps[:], ps[:], bias_sb["bo"][et, :].reshape([128, 1]))
                        yb = bf.tile([128, TBLK], BF16, tag="yx", bufs=HDCH + 4)
                        nc.scalar.copy(yb[:], ps[:])
                        y_bf.append(yb)
                        ysq = bf.tile([128, TBLK], BF16, tag="ysq", bufs=2)
                        nc.scalar.activation(ysq[:], ps[:], AF.Square)
                        if et == 0:
                            nc.vector.tensor_copy(sq_acc[:], ysq[:])
                        else:
                            nc.vector.tensor_add(sq_acc[:], sq_acc[:], ysq[:])
                    sq_bf = bf.tile([128, TBLK], BF16, tag="sqbf", bufs=1)
                    nc.scalar.copy(sq_bf[:], sq_acc[:])
                    ps_ss = bpsS.tile([128, TBLK], FP32, tag="pSt", name="ps_ss")
                    nc.tensor.matmul(ps_ss[:], ones_sb[:], sq_bf[:],
                                     start=True, stop=True)
                    sqs = bf.tile([128, TBLK], FP32, tag="sqs", bufs=1)
                    nc.scalar.activation(sqs[:], ps_ss[:], AF.Sqrt,
                                         scale=1.0 / E, bias=eps_sb[:])
                    rstd = bf.tile([128, TBLK], FP32, tag="rstd", bufs=1)
                    nc.vector.reciprocal_approx_fast(rstd[:], sqs[:])
                    # ---- x1 = x + rmsnorm1(y) (fused) ----
                    x1_t = []
                    for et in range(HDCH):
                        xs = bf.tile([128, TBLK], BF16, tag="xs", bufs=2)
                        nc.sync.dma_start(xs[:], XSL[et * 128:(et + 1) * 128,
                                                     ts0:ts0 + TBLK])
                        x1 = bf.tile([128, TBLK], BF16, tag="yx", bufs=HDCH + 4)
                        n1 = bf.tile([128, TBLK], BF16, tag="n1t", bufs=2)
                        nc.vector.tensor_mul(n1[:], y_bf[et][:], rstd[:])
                        if flags["aw"]:
                            nc.vector.tensor_scalar_mul(
                                n1[:], n1[:], bias_sb["aw"][et, :].reshape([128, 1]))
                        nc.vector.tensor_add(x1[:], xs[:], n1[:])
                        x1_t.append(x1)
                    # ---- FFN up + gate ----
                    h_t = []
                    for ft in range(FCH):
                        ps1 = bpsB.tile([128, TBLK], FP32, tag="pB", name="ps_h1")
                        ps3 = bpsB.tile([128, TBLK], FP32, tag="pB", name="ps_h3")
                        w1_t = bf.tile([128, ECH, 128], BF16, tag="w1", bufs=2)
                        nc.sync.dma_start(
                            w1_t[:],
                            W1[:, ft * 128:(ft + 1) * 128].rearrange(
                                "(c p) f -> p c f", p=128))
                        for ec in range(ECH):
                            nc.tensor.matmul(ps1[:], w1_t[:, ec, :], x1_t[ec][:],
                                             start=(ec == 0), stop=(ec == ECH - 1))
                        w3_t = bf.tile([128, ECH, 128], BF16, tag="w3", bufs=2)
                        nc.sync.dma_start(
                            w3_t[:],
                            W3[:, ft * 128:(ft + 1) * 128].rearrange(
                                "(c p) f -> p c f", p=128))
                        for ec in range(ECH):
                            nc.tensor.matmul(ps3[:], w3_t[:, ec, :], x1_t[ec][:],
                                             start=(ec == 0), stop=(ec == ECH - 1))
                        g1 = bf.tile([128, TBLK], BF16, tag="g1", bufs=2)
                        if flags["b13"]:
                            nc.scalar.activation(
                                g1[:], ps1[:], AF.Gelu,
                                bias=bias_sb["b1"][ft, :].reshape([128, 1]))
                        else:
                            nc.scalar.activation(g1[:], ps1[:], AF.Gelu)
                        ht = bf.tile([128, TBLK], BF16, tag="h", bufs=FCH)
                        if flags["b13"]:
                            nc.vector.scalar_tensor_tensor(
                                ht[:], ps3[:], bias_sb["b3"][ft, :].reshape([128, 1]),
                                g1[:], op0=ALU.add, op1=ALU.mult)
                        else:
                            nc.vector.tensor_mul(ht[:], ps3[:], g1[:])
                        h_t.append(ht)
                    # ---- FFN down + rmsnorm2 + out ----
                    sq_acc2 = bf.tile([128, TBLK], FP32, tag="sqacc2", bufs=1)
                    y2_bf = []
                    for et in range(HDCH):
                        ps = bpsB.tile([128, TBLK], FP32, tag="pB", name="ps_y2")
                        for half in range(2):
                            w2_t = bf.tile([128, FCH // 2, 128], BF16, tag="w2", bufs=2)
                            nc.sync.dma_start(
                                w2_t[:],
                                W2[half * (c.F // 2):(half + 1) * (c.F // 2),
                                   et * 128:(et + 1) * 128].rearrange(
                                    "(q p) e -> p q e", p=128))
                            for fq in range(FCH // 2):
                                fc = half * (FCH // 2) + fq
                                nc.tensor.matmul(
                                    ps[:], w2_t[:, fq, :], h_t[fc][:],
                                    start=(fc == 0), stop=(fc == FCH - 1))
                        if flags["b2"]:
                            nc.vector.tensor_scalar_add(
                                ps[:], ps[:], bias_sb["b2"][et, :].reshape([128, 1]))
                        y2 = bf.tile([128, TBLK], BF16, tag="y2bf", bufs=HDCH + 1)
                        nc.scalar.copy(y2[:], ps[:])
                        y2_bf.append(y2)
                        y2sq = bf.tile([128, TBLK], BF16, tag="y2sq", bufs=2)
                        nc.scalar.activation(y2sq[:], ps[:], AF.Square)
                        if et == 0:
                            nc.vector.tensor_copy(sq_acc2[:], y2sq[:])
                        else:
                            nc.vector.tensor_add(sq_acc2[:], sq_acc2[:], y2sq[:])
                    sq2_bf = bf.tile([128, TBLK], BF16, tag="sq2bf", bufs=1)
                    nc.scalar.copy(sq2_bf[:], sq_acc2[:])
                    ps_ss2 = bpsS.tile([128, TBLK], FP32, tag="pSt", name="ps_ss2")
                    nc.tensor.matmul(ps_ss2[:], ones_sb[:], sq2_bf[:],
                                     start=True, stop=True)
                    sqs2 = bf.tile([128, TBLK], FP32, tag="sqs2", bufs=1)
                    nc.scalar.activation(sqs2[:], ps_ss2[:], AF.Sqrt,
                                         scale=1.0 / E, bias=eps_sb[:])
                    rstd2 = bf.tile([128, TBLK], FP32, tag="rstd2", bufs=1)
                    nc.vector.reciprocal_approx_fast(rstd2[:], sqs2[:])
                    for et in range(HDCH):
                        of = bf.tile([128, TBLK], FP32, tag="of", bufs=2)
                        n2 = bf.tile([128, TBLK], BF16, tag="n2", bufs=2)
                        nc.vector.tensor_mul(n2[:], y2_bf[et][:], rstd2[:])
                        if flags["fw"]:
                            nc.vector.tensor_scalar_mul(
                                n2[:], n2[:], bias_sb["fw"][et, :].reshape([128, 1]))
                        nc.vector.tensor_add(of[:], x1_t[et][:], n2[:])
                        nc.gpsimd.dma_start(OUT[et * 128:(et + 1) * 128, ts0:ts0 + TBLK],
                                          of[:])
            b1_cm.__exit__(None, None, None)
    return tc


def _prep_inputs(inputs, cfg):
    """Host-side: slice/transpose/cast per core. Returns (in_maps, g, flags)."""
    c = cfg
    f32 = np.float32
    x = np.ascontiguousarray(np.asarray(inputs["x"], f32).reshape(c.BS, c.E))
    beta = float(np.asarray(inputs["beta"]).reshape(-1)[0])
    g = 1.0 / (1.0 + math.exp(-beta))
    g = min(max(g, 1e-6), 1.0 - 1e-6)

    Wq = np.asarray(inputs["Wq"], f32)
    Wk = np.asarray(inputs["Wk"], f32)
    Wv = np.asarray(inputs["Wv"], f32)
    Wo = np.asarray(inputs["Wo"], f32)
    W1 = np.asarray(inputs["W1"], f32)
    W2 = np.asarray(inputs["W2"], f32)
    W3 = np.asarray(inputs["W3"], f32)
    aw = np.asarray(inputs["attn_norm_w"], f32)
    fw = np.asarray(inputs["ffn_norm_w"], f32)
    bq = np.asarray(inputs["bq"], f32)
    bk = np.asarray(inputs["bk"], f32)
    bv = np.asarray(inputs["bv"], f32)
    bo = np.asarray(inputs["bo"], f32)
    b1 = np.asarray(inputs["b1"], f32)
    b2 = np.asarray(inputs["b2"], f32)
    b3 = np.asarray(inputs["b3"], f32)

    flags = {
        "bqkv": bool(np.any(bq) or np.any(bk) or np.any(bv)),
        "bo": bool(np.any(bo)),
        "b13": bool(np.any(b1) or np.any(b3)),
        "b2": bool(np.any(b2)),
        "aw": not bool(np.all(aw == 1.0)),
        "fw": not bool(np.all(fw == 1.0)),
    }

    xT = np.ascontiguousarray(x.T).astype(NPBF16)
    woT = np.ascontiguousarray(Wo.T).astype(NPBF16)
    w1T = np.ascontiguousarray(W1.T).astype(NPBF16)
    w3T = np.ascontiguousarray(W3.T).astype(NPBF16)
    w2T = np.ascontiguousarray(W2.T).astype(NPBF16)

    in_maps = []
    for core in range(N_CORES):
        ds = slice(core * c.DPC, (core + 1) * c.DPC)
        tok = slice(core * c.TSL, (core + 1) * c.TSL)
        m = {
            "xT": xT,
            "xslb": np.ascontiguousarray(x[tok].T).astype(NPBF16),
            "wq": np.ascontiguousarray(Wq[ds].T).astype(NPBF16),
            "wk": np.ascontiguousarray(Wk[ds].T).astype(NPBF16),
            "wv": np.ascontiguousarray(Wv[ds].T).astype(NPBF16),
            "wo": woT, "w1": w1T, "w3": w3T, "w2": w2T,
        }
        if flags["bqkv"]:
            m["bq"] = np.tile(bq[ds][None, :], (128, 1)).astype(f32)
            m["bk"] = np.tile(bk[ds][None, :], (128, 1)).astype(f32)
            m["bv"] = np.tile(bv[ds][None, :], (128, 1)).astype(f32)
        if flags["bo"]:
            m["bo"] = bo.reshape(c.E // 128, 128).astype(f32)
        if flags["b13"]:
            m["b1"] = b1.reshape(c.FCH, 128).astype(f32)
            m["b3"] = b3.reshape(c.FCH, 128).astype(f32)
        if flags["b2"]:
            m["b2"] = b2.reshape(c.E // 128, 128).astype(f32)
        if flags["aw"]:
            m["aw"] = aw.reshape(c.E // 128, 128).astype(f32)
        if flags["fw"]:
            m["fw"] = fw.reshape(c.E // 128, 128).astype(f32)
        in_maps.append(m)
    return in_maps, g, flags


def run(inputs, cfg, trace=False):
    in_maps, g, flags = _prep_inputs(inputs, cfg)
    nc = bacc.Bacc("TRN2", target_bir_lowering=False, debug=False,
                   num_devices=N_CORES)
    build(nc, cfg, g, flags)
    nc.compile()
    res = run_bass_kernel_spmd(nc, in_maps, list(range(N_CORES)), trace=trace)
    out = np.empty((cfg.BS, cfg.E), np.float32)
    for core in range(N_CORES):
        out[core * cfg.TSL:(core + 1) * cfg.TSL] = res.results[core]["outT"].T
    return out.reshape(cfg.B, cfg.S, cfg.E), res


def kernel(**inputs):
    out, _ = run(inputs, FULL)
    return out
